# revision 14
# baseline (speedup 1.0000x reference)
"""Trainium2 Bass kernel for nn_ContextAttention (dense transformer block).

Sharding (8 NeuronCores): data-parallel over batch (B=4) x tensor-parallel
over heads (16 heads -> 2 groups of 8). Core c handles batch c//2, head
group c%2. Each core computes QKV projections for its 512 output dims,
the mean-pooled context projections, sequence gating, attention scores in
both orientations ([q,k] for the softmax/p_attn output and [k,q] for the
probs @ V contraction - avoids any on-chip transpose of the 1Mx8 prob
matrices), unnormalized softmax via ScalarE exp with fused row-sum
accumulation, and its slice of the output projection. The two cores of a
batch pair each emit a partial [S,E] output; the host sums the pair and
adds the output bias (the "all-reduce" of the output projection), and
concatenates the per-core [8,S,S] attention-prob slices.

Matmuls stream as float32r (fp32 data, fast PE mode); everything else fp32.
"""

import sys

for _p in ("/opt/trn_rl_repo", "/root/.axon_site/_ro/trn_rl_repo"):
    if _p not in sys.path:
        sys.path.append(_p)

from contextlib import ExitStack

import numpy as np

import concourse.bass as bass  # noqa: F401
import concourse.mybir as mybir
import concourse.tile as tile
from concourse import bacc
from concourse.bass import ts
from concourse.bass_utils import run_bass_kernel_spmd
from concourse.masks import make_identity

F32 = mybir.dt.float32
F32R = mybir.dt.float32r
AF = mybir.ActivationFunctionType
ALU = mybir.AluOpType

S = 1024   # sequence length
B = 4      # batch
E = 1024   # embed dim
H = 16     # total heads
D = 64     # head dim
HPC = 8    # heads per core
O = HPC * D  # 512 local projection dims per core
NCORES = 8
SCALE = 0.125  # 1/sqrt(D)



def build_program(cq: float, ck: float):
    """Build the single SPMD Bass program (same program, per-core data)."""
    nc = bacc.Bacc(
        "TRN2",
        target_bir_lowering=False,
        debug=False,
        num_devices=NCORES,
    )

    # ---- DRAM I/O (per-core data) ----
    xb = nc.dram_tensor("xb", [S, E], F32, kind="ExternalInput").ap()
    mfr = nc.dram_tensor("mfr", [1, S], F32, kind="ExternalInput").ap()
    wqt = nc.dram_tensor("wqt", [E, O], F32R, kind="ExternalInput").ap()
    wkt = nc.dram_tensor("wkt", [E, O], F32R, kind="ExternalInput").ap()
    wvt = nc.dram_tensor("wvt", [E, O], F32R, kind="ExternalInput").ap()
    wgqt = nc.dram_tensor("wgqt", [E, O], F32R, kind="ExternalInput").ap()
    wgkt = nc.dram_tensor("wgkt", [E, O], F32R, kind="ExternalInput").ap()
    bqc = nc.dram_tensor("bqc", [O, 1], F32, kind="ExternalInput").ap()
    bkc = nc.dram_tensor("bkc", [O, 1], F32, kind="ExternalInput").ap()
    bvr = nc.dram_tensor("bvr", [1, O], F32, kind="ExternalInput").ap()
    bgqc = nc.dram_tensor("bgqc", [O, 1], F32, kind="ExternalInput").ap()
    bgkc = nc.dram_tensor("bgkc", [O, 1], F32, kind="ExternalInput").ap()
    wot = nc.dram_tensor("wot", [O, E], F32R, kind="ExternalInput").ap()
    bd_in = {nm: nc.dram_tensor(nm, [O, 97], F32R, kind="ExternalInput").ap()
             for nm in ("bdqA", "bdqB", "bdkA", "bdkB",
                        "bdq2A", "bdq2B", "bdk2A", "bdk2B")}

    p_out = nc.dram_tensor("p_out", [HPC, S, S], F32, kind="ExternalOutput").ap()
    o_out = nc.dram_tensor("o_out", [S, E], F32, kind="ExternalOutput").ap()

    with tile.TileContext(nc) as tc, ExitStack() as stk:
        persist = stk.enter_context(tc.tile_pool(name="persist", bufs=1))
        bigps = stk.enter_context(tc.tile_pool(name="bigps", bufs=2, space="PSUM"))
        smallps = stk.enter_context(tc.tile_pool(name="smallps", bufs=2, space="PSUM"))
        ctxps = stk.enter_context(tc.tile_pool(name="ctxps", bufs=1, space="PSUM"))
        dscr = stk.enter_context(tc.tile_pool(name="dscr", bufs=2, space="DRAM"))

        ident = persist.tile([128, 128], F32, name="ident")
        make_identity(nc, ident[:])

        # ============ Phase A: load x, transpose, mean-pool g ============
        g_sb = persist.tile([128, 8], F32R, name="g_sb")
        g_f32 = persist.tile([128, 8], F32, name="g_f32")
        v_sb = [persist.tile([128, O], F32R, name=f"v{i}") for i in range(8)]
        gq_sb = persist.tile([128, 4], F32R, name="gq_sb")
        gk_sb = persist.tile([128, 4], F32R, name="gk_sb")
        qd_sb = [persist.tile([128, 1], F32, name=f"qd_sb{i}") for i in range(2)]
        kd_sb = [persist.tile([128, 1], F32, name=f"kd_sb{i}") for i in range(2)]
        bd_sb = {nm: [persist.tile([128, 97], F32R, name=f"{nm}_{i}")
                      for i in range(4)]
                 for nm in ("bdqA", "bdqB", "bdkA", "bdkB")}
        ones_col = persist.tile([128, 2], F32R, name="ones_col")
        ones_f32 = persist.tile([128, 2], F32, name="ones_f32")
        nc.vector.memset(ones_f32[:], 1.0)
        nc.vector.tensor_copy(ones_col[:], ones_f32[:])
        qhT = [persist.tile([128, S], F32R, name=f"qhT{j}") for j in range(4)]
        khT = [persist.tile([128, S], F32R, name=f"khT{j}") for j in range(4)]

        with tc.tile_pool(name="qkvp", bufs=1) as qkvp:
            qT = [qkvp.tile([128, S], F32R, name=f"qT{j}") for j in range(4)]
            kT = [qkvp.tile([128, S], F32R, name=f"kT{j}") for j in range(4)]

            with tc.tile_pool(name="xtp", bufs=1) as xtp:
                xT = [xtp.tile([128, S], F32R, name=f"xT{j}") for j in range(8)]

                with tc.tile_pool(name="phA", bufs=8) as pha, \
                        tc.tile_pool(name="phA1", bufs=1) as pha1:
                    x_nat = []
                    for i in range(8):
                        t = pha.tile([128, E], F32, name="x_nat", tag="x_nat")
                        nc.sync.dma_start(t[:], xb[ts(i, 128), :])
                        x_nat.append(t)
                    mf_bc = pha1.tile([128, S], F32, name="mf_bc")
                    nc.sync.dma_start(mf_bc[:], mfr[0:1, :].to_broadcast((128, S)))

                    for j in range(8):
                        psx = bigps.tile([128, S], F32, name="psx", tag="bigtile")
                        for i in range(8):
                            nc.tensor.transpose(
                                psx[:, ts(i, 128)], x_nat[i][:, ts(j, 128)], ident[:]
                            )
                        nc.vector.tensor_copy(xT[j][:], psx[:])
                    gscr = pha1.tile([128, S], F32, name="gscr")
                    for j in range(8):
                        nc.vector.scalar_tensor_tensor(
                            gscr[:], xT[j][:].bitcast(F32), 1.0, mf_bc[:],
                            op0=ALU.mult, op1=ALU.mult,
                            accum_out=g_f32[:, j:j + 1],
                        )
                    nc.vector.tensor_copy(g_sb[:], g_f32[:])

                # ======== Phase B: projections ========
                with tc.tile_pool(name="phB", bufs=1) as phb:
                    bq_sb = phb.tile([128, 4], F32, name="bq_sb")
                    bk_sb = phb.tile([128, 4], F32, name="bk_sb")
                    bgq_sb = phb.tile([128, 4], F32, name="bgq_sb")
                    bgk_sb = phb.tile([128, 4], F32, name="bgk_sb")
                    for oc in range(4):
                        nc.sync.dma_start(bq_sb[:, oc:oc + 1], bqc[ts(oc, 128), :])
                        nc.sync.dma_start(bk_sb[:, oc:oc + 1], bkc[ts(oc, 128), :])
                        nc.sync.dma_start(bgq_sb[:, oc:oc + 1], bgqc[ts(oc, 128), :])
                        nc.sync.dma_start(bgk_sb[:, oc:oc + 1], bgkc[ts(oc, 128), :])
                    bv_sb = phb.tile([1, O], F32, name="bv_sb")
                    nc.sync.dma_start(bv_sb[:], bvr[:, :])
                    ones1 = phb.tile([1, 128], F32, name="ones1")
                    nc.vector.memset(ones1[:], 1.0)
                    for nm in ("bdqA", "bdqB", "bdkA", "bdkB"):
                        for i in range(4):
                            nc.sync.dma_start(bd_sb[nm][i][:], bd_in[nm][ts(i, 128), :])
                    bd2_sb = {nm: [phb.tile([128, 97], F32R, name=f"{nm}_{i}")
                                   for i in range(4)]
                              for nm in ("bdq2A", "bdq2B", "bdk2A", "bdk2B")}
                    for nm in ("bdq2A", "bdq2B", "bdk2A", "bdk2B"):
                        for i in range(4):
                            nc.sync.dma_start(bd2_sb[nm][i][:], bd_in[nm][ts(i, 128), :])

                    def load_wset(src_ap):
                        tiles = []
                        for ke in range(8):
                            t = phb.tile([128, O], F32R, name="wmat", tag="wmat",
                                         bufs=16)
                            nc.sync.dma_start(t[:], src_ap[ts(ke, 128), :])
                            tiles.append(t)
                        return tiles

                    # q^T, k^T projections: [O, S] as 4 tiles [128, 1024]
                    for dst, wsrc, bias in ((qT, wqt, bq_sb), (kT, wkt, bk_sb)):
                        wsb = load_wset(wsrc)
                        for oc in range(4):
                            ps = bigps.tile([128, S], F32, name="psB", tag="bigtile")
                            for tch in range(2):
                                for ke in range(8):
                                    nc.tensor.matmul(
                                        ps[:, ts(tch, 512)],
                                        (wsb[ke][:, ts(oc, 128)]),
                                        (xT[ke][:, ts(tch, 512)]),
                                        start=(ke == 0), stop=(ke == 7),
                                    )
                            nc.vector.tensor_scalar_add(dst[oc][:], ps[:], bias[:, oc:oc + 1])

                    # v: [S, O] natural layout, bias folded via rank-1 matmul
                    wsb = load_wset(wvt)
                    for tt in range(8):
                        psv = bigps.tile([128, O], F32, name="psv", tag="bigtile")
                        for ke in range(8):
                            nc.tensor.matmul(
                                psv[:, :],
                                (xT[ke][:, ts(tt, 128)]),
                                (wsb[ke][:]),
                                start=(ke == 0), stop=False,
                            )
                        nc.tensor.matmul(
                            psv[:, :], ones1[:], bv_sb[:],
                            start=False, stop=True,
                        )
                        nc.vector.tensor_copy(v_sb[tt][:], psv[:, :])

                    # context projections gq, gk: [O, 1]
                    for dst, wsrc, bias in ((gq_sb, wgqt, bgq_sb), (gk_sb, wgkt, bgk_sb)):
                        wsb = load_wset(wsrc)
                        for oc in range(4):
                            psg = smallps.tile([128, 1], F32, name="psg", tag="smalltile")
                            for ke in range(8):
                                nc.tensor.matmul(
                                    psg[:, :],
                                    wsb[ke][:, ts(oc, 128)].bitcast(F32),
                                    g_sb[:, ke:ke + 1].bitcast(F32),
                                    start=(ke == 0), stop=(ke == 7),
                                )
                            nc.vector.tensor_scalar_add(
                                dst[:, oc:oc + 1], psg[:, :], bias[:, oc:oc + 1]
                            )

                    # per-head gate constants (heads spread to partitions 0/32/64/96)
                    for dsts, nms, gcol, const in (
                        (qd_sb, ("bdq2A", "bdq2B"), gq_sb, cq),
                        (kd_sb, ("bdk2A", "bdk2B"), gk_sb, ck),
                    ):
                        for grp in range(2):
                            psd = smallps.tile([128, 1], F32, name="psd", tag="smalltile")
                            for kc in range(4):
                                nc.tensor.matmul(
                                    psd[0:97, :],
                                    bd2_sb[nms[grp]][kc][:].bitcast(F32),
                                    gcol[:, kc:kc + 1].bitcast(F32),
                                    start=(kc == 0), stop=(kc == 3),
                                )
                            nc.vector.tensor_scalar_add(
                                dsts[grp][0:97, :], psd[0:97, :], float(const))

            # ======== Phase C: sequence gating (xT freed) ========
            with tc.tile_pool(name="phC", bufs=2) as phc, \
                    tc.tile_pool(name="phCa", bufs=2) as phca:
                for src, dst, nms, gcol, dcols in (
                    (qT, qhT, ("bdqA", "bdqB"), gq_sb, qd_sb),
                    (kT, khT, ("bdkA", "bdkB"), gk_sb, kd_sb),
                ):
                    alphas = []
                    for grp in range(2):
                        psa = bigps.tile([128, S], F32, name="psa", tag="bigtile")
                        for tch in range(2):
                            for kc in range(4):
                                nc.tensor.matmul(
                                    psa[0:97, ts(tch, 512)],
                                    (bd_sb[nms[grp]][kc][:]),
                                    (src[kc][:, ts(tch, 512)]),
                                    start=(kc == 0), stop=(kc == 3),
                                )
                        alpha = phca.tile([128, S], F32, name="alpha", tag="alpha")
                        nc.scalar.activation(alpha[0:97, :], psa[0:97, :], AF.Sigmoid,
                                             bias=dcols[grp][0:97, :])
                        alphas.append(alpha)
                    alpha_d = dscr.tile([8, S], F32, name="alpha_d", tag="alpha_d")
                    for grp in range(2):
                        nc.sync.dma_start(alpha_d[4 * grp:4 * grp + 4, :],
                                          alphas[grp][0:128:32, :])
                    for j in range(4):
                        abc = phc.tile([128, S], F32, name="abc", tag="abc")
                        for hh in range(2):
                            h = 2 * j + hh
                            nc.sync.dma_start(
                                abc[64 * hh:64 * hh + 64, :],
                                alpha_d[h:h + 1, :].to_broadcast((64, S)))
                        tmpg = phc.tile([128, S], F32, name="tmpg", tag="tmpg")
                        nc.vector.scalar_tensor_tensor(
                            tmpg[:], src[j][:].bitcast(F32), gcol[:, j:j + 1].bitcast(F32), abc[:],
                            op0=ALU.subtract, op1=ALU.mult,
                        )
                        nc.vector.tensor_sub(dst[j][:], src[j][:].bitcast(F32), tmpg[:])

        # ============ Phase D: attention per head pair ============
        den_all = persist.tile([128, 8 * HPC], F32, name="den_all")
        rec_all = persist.tile([128, 8 * HPC], F32, name="rec_all")
        ctxT = [persist.tile([128, S], F32R, name=f"ctxT{j}") for j in range(4)]
        # fp32r matmuls require partition-base-0 operands: stage odd heads
        # (partitions 64..127 of the packed q/k tiles) into base-0 tiles.
        with tc.tile_pool(name="phD_qk", bufs=1) as pqk, \
                tc.tile_pool(name="phD_eT", bufs=9) as peT, \
                tc.tile_pool(name="phD_e", bufs=9) as pe_, \
                tc.tile_pool(name="phD_p", bufs=3) as pp, \
                tc.tile_pool(name="phD_r", bufs=2) as pr, \
                tc.tile_pool(name="phD_cs", bufs=1) as pcs:
            qh_odd = [pqk.tile([64, S], F32R, name=f"qho{j}") for j in range(4)]
            kh_odd = [pqk.tile([64, S], F32R, name=f"kho{j}") for j in range(4)]
            for j in range(4):
                nc.sync.dma_start(qh_odd[j][:], qhT[j][64:128, :])
                nc.sync.dma_start(kh_odd[j][:], khT[j][64:128, :])
            for hp in range(4):
                for hh in range(2):
                    h = 2 * hp + hh
                    if hh == 0:
                        qh = qhT[hp][0:64, :]
                        kh = khT[hp][0:64, :]
                    else:
                        qh = qh_odd[hp][:, :]
                        kh = kh_odd[hp][:, :]

                    # scores [k, q] -> exp -> eT tiles (for probs @ V)
                    eT = []
                    for kt in range(8):
                        psT = bigps.tile([128, S], F32, name="psT", tag="bigtile")
                        for qch in range(2):
                            nc.tensor.matmul(
                                psT[:, ts(qch, 512)],
                                (kh[:, ts(kt, 128)]),
                                (qh[:, ts(qch, 512)]),
                                start=True, stop=True,
                            )
                        et = peT.tile([128, S], F32R, name="eT", tag="eT")
                        nc.scalar.activation(et[:], psT[:], AF.Exp, scale=SCALE)
                        eT.append(et)

                    # scores [q, k] -> exp with row-sum accum -> e tiles
                    e_tiles = []
                    for qt in range(8):
                        psS = bigps.tile([128, S], F32, name="psS", tag="bigtile")
                        for kch in range(2):
                            nc.tensor.matmul(
                                psS[:, ts(kch, 512)],
                                (qh[:, ts(qt, 128)]),
                                (kh[:, ts(kch, 512)]),
                                start=True, stop=True,
                            )
                        et = pe_.tile([128, S], F32, name="e", tag="e")
                        idx = h * 8 + qt
                        nc.scalar.activation(
                            et[:], psS[:], AF.Exp, scale=SCALE,
                            accum_out=den_all[:, idx:idx + 1],
                        )
                        e_tiles.append(et)

                    # normalize p = e / den and store
                    nc.vector.reciprocal(
                        rec_all[:, h * 8:h * 8 + 8], den_all[:, h * 8:h * 8 + 8]
                    )
                    for qt in range(8):
                        pt = pp.tile([128, S], F32, name="p", tag="p")
                        nc.vector.tensor_scalar_mul(
                            pt[:], e_tiles[qt][:],
                            rec_all[:, h * 8 + qt:h * 8 + qt + 1],
                        )
                        nc.sync.dma_start(p_out[h, ts(qt, 128), :], pt[:])

                    # denominator as a [1, S] row via ones-matmul over eT
                    psden = bigps.tile([128, S], F32, name="psden", tag="bigtile")
                    for qch in range(2):
                        for kt in range(8):
                            nc.tensor.matmul(
                                psden[0:2, ts(qch, 512)],
                                (ones_col[:]),
                                (eT[kt][:, ts(qch, 512)]),
                                start=(kt == 0), stop=(kt == 7),
                            )
                    rec_row = pr.tile([1, S], F32, name="rec_row", tag="rec_row")
                    nc.vector.reciprocal(rec_row[0:1, :], psden[0:1, :])
                    recd = dscr.tile([1, S], F32, name="recd", tag="recd")
                    nc.sync.dma_start(recd[0:1, :], rec_row[0:1, :])
                    rbc = pr.tile([64, S], F32, name="rbc", tag="rbc")
                    nc.sync.dma_start(rbc[:, :], recd[0:1, :].to_broadcast((64, S)))

                    # ctx^T[d, q] = v_h^T @ eT (unnormalized), base-0 psum
                    psc = ctxps.tile([64, S], F32, name="psC", tag="psC")
                    for qch in range(2):
                        for kt in range(8):
                            nc.tensor.matmul(
                                psc[:, ts(qch, 512)],
                                (v_sb[kt][:, ts(h, 64)]),
                                (eT[kt][:, ts(qch, 512)]),
                                start=(kt == 0), stop=(kt == 7),
                            )
                    if hh == 0:
                        nc.vector.tensor_tensor(
                            ctxT[hp][0:64, :], psc[:, :], rbc[:], op=ALU.mult)
                    else:
                        csc = pcs.tile([64, S], F32R, name="csc", tag="csc")
                        nc.vector.tensor_tensor(
                            csc[:], psc[:, :], rbc[:], op=ALU.mult)
                        nc.sync.dma_start(ctxT[hp][64:128, :], csc[:])

        # ============ Phase E: output projection (partial) ============
        with tc.tile_pool(name="phE", bufs=1) as phe, \
                tc.tile_pool(name="phEo", bufs=2) as pheo:
            wot_sb = [phe.tile([128, E], F32R, name=f"wot{i}") for i in range(4)]
            for i in range(4):
                nc.sync.dma_start(wot_sb[i][:], wot[ts(i, 128), :])
            for st in range(8):
                pso = bigps.tile([128, E], F32, name="psO", tag="bigtile")
                for ech in range(2):
                    for kc in range(4):
                        nc.tensor.matmul(
                            pso[:, ts(ech, 512)],
                            (ctxT[kc][:, ts(st, 128)]),
                            (wot_sb[kc][:, ts(ech, 512)]),
                            start=(kc == 0), stop=(kc == 3),
                        )
                osb = pheo.tile([128, E], F32, name="osb", tag="osb")
                nc.vector.tensor_copy(osb[:], pso[:])
                nc.sync.dma_start(o_out[ts(st, 128), :], osb[:])

    nc.compile()
    return nc


_PROGRAM_CACHE = {}


def _get_program(cq, ck):
    key = (round(float(cq), 10), round(float(ck), 10), "f32r")
    if key not in _PROGRAM_CACHE:
        _PROGRAM_CACHE[key] = build_program(float(cq), float(ck))
    return _PROGRAM_CACHE[key]


def make_in_maps(inputs):
    """Host-side sharding: build the 8 per-core input dicts."""
    x = np.asarray(inputs["x"], np.float32)
    mask = np.asarray(inputs["mask"])
    in_maps = []
    for c in range(NCORES):
        b, g = divmod(c, 2)
        hs = slice(g * O, (g + 1) * O)
        maskf = mask[b].astype(np.float32)
        msum = max(float(maskf.sum()), 1e-9)
        bds = {nm: np.zeros((O, 97), np.float32)
               for nm in ("bdqA", "bdqB", "bdkA", "bdkB",
                          "bdq2A", "bdq2B", "bdk2A", "bdk2B")}
        for h in range(HPC):
            sfx, col = ("A", 32 * h) if h < 4 else ("B", 32 * (h - 4))
            rows = slice(h * D, (h + 1) * D)
            bds["bdq" + sfx][rows, col] = inputs["qg_wq"][0]
            bds["bdk" + sfx][rows, col] = inputs["kg_wq"][0]
            bds["bdq2" + sfx][rows, col] = inputs["qg_wk"][0]
            bds["bdk2" + sfx][rows, col] = inputs["kg_wk"][0]
        im = dict(
            xb=np.ascontiguousarray(x[:, b, :]),
            mfr=np.ascontiguousarray((maskf / msum)[None, :]),
            wqt=np.ascontiguousarray(inputs["Wq"][hs].T),
            wkt=np.ascontiguousarray(inputs["Wk"][hs].T),
            wvt=np.ascontiguousarray(inputs["Wv"][hs].T),
            wgqt=np.ascontiguousarray(inputs["Wgq"][hs].T),
            wgkt=np.ascontiguousarray(inputs["Wgk"][hs].T),
            bqc=np.ascontiguousarray(inputs["bq"][hs][:, None]),
            bkc=np.ascontiguousarray(inputs["bk"][hs][:, None]),
            bvr=np.ascontiguousarray(inputs["bv"][hs][None, :]),
            bgqc=np.ascontiguousarray(inputs["bgq"][hs][:, None]),
            bgkc=np.ascontiguousarray(inputs["bgk"][hs][:, None]),
            wot=np.ascontiguousarray(inputs["Wo"][:, hs].T),
            **bds,
        )
        im = {k: np.ascontiguousarray(v, np.float32) for k, v in im.items()}
        in_maps.append(im)
    return in_maps


def gather_outputs(inputs, results):
    out = np.zeros((S, B, E), np.float32)
    p_attn = np.empty((B, H, S, S), np.float32)
    bo = np.asarray(inputs["bo"], np.float32)
    for c in range(NCORES):
        b, g = divmod(c, 2)
        out[:, b, :] += results[c]["o_out"]
        p_attn[b, g * HPC:(g + 1) * HPC] = results[c]["p_out"]
    out += bo[None, None, :]
    return out, p_attn


def kernel(**inputs):
    cq = float(np.asarray(inputs["qg_bq"]).reshape(-1)[0]
               + np.asarray(inputs["qg_bk"]).reshape(-1)[0])
    ck = float(np.asarray(inputs["kg_bq"]).reshape(-1)[0]
               + np.asarray(inputs["kg_bk"]).reshape(-1)[0])
    nc = _get_program(cq, ck)
    in_maps = make_in_maps(inputs)
    res = run_bass_kernel_spmd(nc, in_maps, core_ids=list(range(NCORES)))
    return gather_outputs(inputs, res.results)


# revision 15
# speedup vs baseline: 1.0900x; 1.0900x over previous
"""Trainium2 Bass kernel for nn_ContextAttention (dense transformer block).

Sharding (8 NeuronCores): data-parallel over batch (B=4) x tensor-parallel
over heads (16 heads -> 2 groups of 8). Core c handles batch c//2, head
group c%2. Each core computes QKV projections for its 512 output dims,
the mean-pooled context projections, sequence gating, attention scores in
both orientations ([q,k] for the softmax/p_attn output and [k,q] for the
probs @ V contraction - avoids any on-chip transpose of the 1Mx8 prob
matrices), unnormalized softmax via ScalarE exp with fused row-sum
accumulation, and its slice of the output projection. The two cores of a
batch pair each emit a partial [S,E] output; the host sums the pair and
adds the output bias (the "all-reduce" of the output projection), and
concatenates the per-core [8,S,S] attention-prob slices.

Matmuls stream as float32r (fp32 data, fast PE mode); everything else fp32.
"""

import sys

for _p in ("/opt/trn_rl_repo", "/root/.axon_site/_ro/trn_rl_repo"):
    if _p not in sys.path:
        sys.path.append(_p)

from contextlib import ExitStack

import numpy as np

import concourse.bass as bass  # noqa: F401
import concourse.mybir as mybir
import concourse.tile as tile
from concourse import bacc
from concourse.bass import ts
from concourse.bass_utils import run_bass_kernel_spmd
from concourse.masks import make_identity

F32 = mybir.dt.float32
F32R = mybir.dt.float32r
AF = mybir.ActivationFunctionType
ALU = mybir.AluOpType

S = 1024   # sequence length
B = 4      # batch
E = 1024   # embed dim
H = 16     # total heads
D = 64     # head dim
HPC = 8    # heads per core
O = HPC * D  # 512 local projection dims per core
NCORES = 8
SCALE = 0.125  # 1/sqrt(D)



def build_program(cq: float, ck: float):
    """Build the single SPMD Bass program (same program, per-core data)."""
    nc = bacc.Bacc(
        "TRN2",
        target_bir_lowering=False,
        debug=False,
        num_devices=NCORES,
    )

    # ---- DRAM I/O (per-core data) ----
    xb = nc.dram_tensor("xb", [S, E], F32, kind="ExternalInput").ap()
    mfr = nc.dram_tensor("mfr", [1, S], F32, kind="ExternalInput").ap()
    wqt = nc.dram_tensor("wqt", [E, O], F32R, kind="ExternalInput").ap()
    wkt = nc.dram_tensor("wkt", [E, O], F32R, kind="ExternalInput").ap()
    wvt = nc.dram_tensor("wvt", [E, O], F32R, kind="ExternalInput").ap()
    wgqt = nc.dram_tensor("wgqt", [E, O], F32R, kind="ExternalInput").ap()
    wgkt = nc.dram_tensor("wgkt", [E, O], F32R, kind="ExternalInput").ap()
    bqc = nc.dram_tensor("bqc", [O, 1], F32, kind="ExternalInput").ap()
    bkc = nc.dram_tensor("bkc", [O, 1], F32, kind="ExternalInput").ap()
    bvr = nc.dram_tensor("bvr", [1, O], F32, kind="ExternalInput").ap()
    bgqc = nc.dram_tensor("bgqc", [O, 1], F32, kind="ExternalInput").ap()
    bgkc = nc.dram_tensor("bgkc", [O, 1], F32, kind="ExternalInput").ap()
    wot = nc.dram_tensor("wot", [O, E], F32R, kind="ExternalInput").ap()
    bd_in = {nm: nc.dram_tensor(nm, [O, 97], F32R, kind="ExternalInput").ap()
             for nm in ("bdqA", "bdqB", "bdkA", "bdkB",
                        "bdq2A", "bdq2B", "bdk2A", "bdk2B")}

    p_out = nc.dram_tensor("p_out", [HPC, S, S], F32, kind="ExternalOutput").ap()
    o_out = nc.dram_tensor("o_out", [S, E], F32, kind="ExternalOutput").ap()

    with tile.TileContext(nc) as tc, ExitStack() as stk:
        persist = stk.enter_context(tc.tile_pool(name="persist", bufs=1))
        bigps = stk.enter_context(tc.tile_pool(name="bigps", bufs=3, space="PSUM"))
        dscr = stk.enter_context(tc.tile_pool(name="dscr", bufs=2, space="DRAM"))

        ident = persist.tile([128, 128], F32, name="ident")
        make_identity(nc, ident[:])

        # ============ Phase A: load x, transpose, mean-pool g ============
        g_sb = persist.tile([128, 8], F32R, name="g_sb")
        g_f32 = persist.tile([128, 8], F32, name="g_f32")
        v_sb = [persist.tile([128, O], F32R, name=f"v{i}") for i in range(8)]
        gq_sb = persist.tile([128, 4], F32R, name="gq_sb")
        gk_sb = persist.tile([128, 4], F32R, name="gk_sb")
        qd_sb = [persist.tile([128, 1], F32, name=f"qd_sb{i}") for i in range(2)]
        kd_sb = [persist.tile([128, 1], F32, name=f"kd_sb{i}") for i in range(2)]
        bd_sb = {nm: [persist.tile([128, 97], F32R, name=f"{nm}_{i}")
                      for i in range(4)]
                 for nm in ("bdqA", "bdqB", "bdkA", "bdkB")}
        ones_col = persist.tile([128, 2], F32R, name="ones_col")
        ones_f32 = persist.tile([128, 2], F32, name="ones_f32")
        nc.vector.memset(ones_f32[:], 1.0)
        nc.vector.tensor_copy(ones_col[:], ones_f32[:])
        qhT = [persist.tile([128, S], F32R, name=f"qhT{j}") for j in range(4)]
        khT = [persist.tile([128, S], F32R, name=f"khT{j}") for j in range(4)]

        with tc.tile_pool(name="qkvp", bufs=1) as qkvp:
            qT = [qkvp.tile([128, S], F32R, name=f"qT{j}") for j in range(4)]
            kT = [qkvp.tile([128, S], F32R, name=f"kT{j}") for j in range(4)]

            with tc.tile_pool(name="xtp", bufs=1) as xtp:
                xT = [xtp.tile([128, S], F32R, name=f"xT{j}") for j in range(8)]

                with tc.tile_pool(name="phA", bufs=8) as pha, \
                        tc.tile_pool(name="phA1", bufs=1) as pha1:
                    x_nat = []
                    for i in range(8):
                        t = pha.tile([128, E], F32, name="x_nat", tag="x_nat")
                        nc.sync.dma_start(t[:], xb[ts(i, 128), :])
                        x_nat.append(t)
                    mf_bc = pha1.tile([128, S], F32, name="mf_bc")
                    nc.sync.dma_start(mf_bc[:], mfr[0:1, :].to_broadcast((128, S)))

                    for j in range(8):
                        psx = bigps.tile([128, S], F32, name="psx", tag="bigtile")
                        for i in range(8):
                            nc.tensor.transpose(
                                psx[:, ts(i, 128)], x_nat[i][:, ts(j, 128)], ident[:]
                            )
                        nc.vector.tensor_copy(xT[j][:], psx[:])
                    gscr = pha1.tile([128, S], F32, name="gscr")
                    for j in range(8):
                        nc.vector.scalar_tensor_tensor(
                            gscr[:], xT[j][:].bitcast(F32), 1.0, mf_bc[:],
                            op0=ALU.mult, op1=ALU.mult,
                            accum_out=g_f32[:, j:j + 1],
                        )
                    nc.vector.tensor_copy(g_sb[:], g_f32[:])

                # ======== Phase B: projections ========
                with tc.tile_pool(name="phB", bufs=1) as phb, \
                        tc.tile_pool(name="smallps", bufs=2,
                                     space="PSUM") as smallps:
                    bq_sb = phb.tile([128, 4], F32, name="bq_sb")
                    bk_sb = phb.tile([128, 4], F32, name="bk_sb")
                    bgq_sb = phb.tile([128, 4], F32, name="bgq_sb")
                    bgk_sb = phb.tile([128, 4], F32, name="bgk_sb")
                    for oc in range(4):
                        nc.sync.dma_start(bq_sb[:, oc:oc + 1], bqc[ts(oc, 128), :])
                        nc.sync.dma_start(bk_sb[:, oc:oc + 1], bkc[ts(oc, 128), :])
                        nc.sync.dma_start(bgq_sb[:, oc:oc + 1], bgqc[ts(oc, 128), :])
                        nc.sync.dma_start(bgk_sb[:, oc:oc + 1], bgkc[ts(oc, 128), :])
                    bv_sb = phb.tile([1, O], F32, name="bv_sb")
                    nc.sync.dma_start(bv_sb[:], bvr[:, :])
                    ones1 = phb.tile([1, 128], F32, name="ones1")
                    nc.vector.memset(ones1[:], 1.0)
                    for nm in ("bdqA", "bdqB", "bdkA", "bdkB"):
                        for i in range(4):
                            nc.sync.dma_start(bd_sb[nm][i][:], bd_in[nm][ts(i, 128), :])
                    bd2_sb = {nm: [phb.tile([128, 97], F32R, name=f"{nm}_{i}")
                                   for i in range(4)]
                              for nm in ("bdq2A", "bdq2B", "bdk2A", "bdk2B")}
                    for nm in ("bdq2A", "bdq2B", "bdk2A", "bdk2B"):
                        for i in range(4):
                            nc.sync.dma_start(bd2_sb[nm][i][:], bd_in[nm][ts(i, 128), :])

                    def load_wset(src_ap):
                        tiles = []
                        for ke in range(8):
                            t = phb.tile([128, O], F32R, name="wmat", tag="wmat",
                                         bufs=16)
                            nc.sync.dma_start(t[:], src_ap[ts(ke, 128), :])
                            tiles.append(t)
                        return tiles

                    # q^T, k^T projections: [O, S] as 4 tiles [128, 1024]
                    for dst, wsrc, bias in ((qT, wqt, bq_sb), (kT, wkt, bk_sb)):
                        wsb = load_wset(wsrc)
                        for oc in range(4):
                            ps = bigps.tile([128, S], F32, name="psB", tag="bigtile")
                            for tch in range(2):
                                for ke in range(8):
                                    nc.tensor.matmul(
                                        ps[:, ts(tch, 512)],
                                        (wsb[ke][:, ts(oc, 128)]),
                                        (xT[ke][:, ts(tch, 512)]),
                                        start=(ke == 0), stop=(ke == 7),
                                    )
                            nc.vector.tensor_scalar_add(dst[oc][:], ps[:], bias[:, oc:oc + 1])

                    # v: [S, O] natural layout, bias folded via rank-1 matmul
                    wsb = load_wset(wvt)
                    for tt in range(8):
                        psv = bigps.tile([128, O], F32, name="psv", tag="bigtile")
                        for ke in range(8):
                            nc.tensor.matmul(
                                psv[:, :],
                                (xT[ke][:, ts(tt, 128)]),
                                (wsb[ke][:]),
                                start=(ke == 0), stop=False,
                            )
                        nc.tensor.matmul(
                            psv[:, :], ones1[:], bv_sb[:],
                            start=False, stop=True,
                        )
                        nc.vector.tensor_copy(v_sb[tt][:], psv[:, :])

                    # context projections gq, gk: [O, 1]
                    for dst, wsrc, bias in ((gq_sb, wgqt, bgq_sb), (gk_sb, wgkt, bgk_sb)):
                        wsb = load_wset(wsrc)
                        for oc in range(4):
                            psg = smallps.tile([128, 1], F32, name="psg", tag="smalltile")
                            for ke in range(8):
                                nc.tensor.matmul(
                                    psg[:, :],
                                    wsb[ke][:, ts(oc, 128)].bitcast(F32),
                                    g_sb[:, ke:ke + 1].bitcast(F32),
                                    start=(ke == 0), stop=(ke == 7),
                                )
                            nc.vector.tensor_scalar_add(
                                dst[:, oc:oc + 1], psg[:, :], bias[:, oc:oc + 1]
                            )

                    # per-head gate constants (heads spread to partitions 0/32/64/96)
                    for dsts, nms, gcol, const in (
                        (qd_sb, ("bdq2A", "bdq2B"), gq_sb, cq),
                        (kd_sb, ("bdk2A", "bdk2B"), gk_sb, ck),
                    ):
                        for grp in range(2):
                            psd = smallps.tile([128, 1], F32, name="psd", tag="smalltile")
                            for kc in range(4):
                                nc.tensor.matmul(
                                    psd[0:97, :],
                                    bd2_sb[nms[grp]][kc][:].bitcast(F32),
                                    gcol[:, kc:kc + 1].bitcast(F32),
                                    start=(kc == 0), stop=(kc == 3),
                                )
                            nc.vector.tensor_scalar_add(
                                dsts[grp][0:97, :], psd[0:97, :], float(const))

            # ======== Phase C: sequence gating (xT freed) ========
            with tc.tile_pool(name="phC", bufs=2) as phc, \
                    tc.tile_pool(name="phCa", bufs=2) as phca:
                for src, dst, nms, gcol, dcols in (
                    (qT, qhT, ("bdqA", "bdqB"), gq_sb, qd_sb),
                    (kT, khT, ("bdkA", "bdkB"), gk_sb, kd_sb),
                ):
                    alphas = []
                    for grp in range(2):
                        psa = bigps.tile([128, S], F32, name="psa", tag="bigtile")
                        for tch in range(2):
                            for kc in range(4):
                                nc.tensor.matmul(
                                    psa[0:97, ts(tch, 512)],
                                    (bd_sb[nms[grp]][kc][:]),
                                    (src[kc][:, ts(tch, 512)]),
                                    start=(kc == 0), stop=(kc == 3),
                                )
                        alpha = phca.tile([128, S], F32, name="alpha", tag="alpha")
                        nc.scalar.activation(alpha[0:97, :], psa[0:97, :], AF.Sigmoid,
                                             bias=dcols[grp][0:97, :])
                        alphas.append(alpha)
                    alpha_d = dscr.tile([8, S], F32, name="alpha_d", tag="alpha_d")
                    for grp in range(2):
                        nc.sync.dma_start(alpha_d[4 * grp:4 * grp + 4, :],
                                          alphas[grp][0:128:32, :])
                    for j in range(4):
                        abc = phc.tile([128, S], F32, name="abc", tag="abc")
                        for hh in range(2):
                            h = 2 * j + hh
                            nc.sync.dma_start(
                                abc[64 * hh:64 * hh + 64, :],
                                alpha_d[h:h + 1, :].to_broadcast((64, S)))
                        tmpg = phc.tile([128, S], F32, name="tmpg", tag="tmpg")
                        nc.vector.scalar_tensor_tensor(
                            tmpg[:], src[j][:].bitcast(F32), gcol[:, j:j + 1].bitcast(F32), abc[:],
                            op0=ALU.subtract, op1=ALU.mult,
                        )
                        nc.vector.tensor_sub(dst[j][:], src[j][:].bitcast(F32), tmpg[:])

        # ============ Phase D: attention per head pair ============
        den_all = persist.tile([128, 8 * HPC], F32, name="den_all")
        rec_all = persist.tile([128, 8 * HPC], F32, name="rec_all")
        ctxT = [persist.tile([128, S], F32R, name=f"ctxT{j}") for j in range(4)]
        # fp32r matmuls require partition-base-0 operands: stage odd heads
        # (partitions 64..127 of the packed q/k tiles) into base-0 tiles.
        with tc.tile_pool(name="phD_qk", bufs=1) as pqk, \
                tc.tile_pool(name="ctxps", bufs=1, space="PSUM") as ctxps, \
                tc.tile_pool(name="phD_eT", bufs=9) as peT, \
                tc.tile_pool(name="phD_e", bufs=9) as pe_, \
                tc.tile_pool(name="phD_p", bufs=3) as pp, \
                tc.tile_pool(name="phD_r", bufs=2) as pr, \
                tc.tile_pool(name="phD_cs", bufs=1) as pcs:
            qh_odd = [pqk.tile([64, S], F32R, name=f"qho{j}") for j in range(4)]
            kh_odd = [pqk.tile([64, S], F32R, name=f"kho{j}") for j in range(4)]
            for j in range(4):
                nc.sync.dma_start(qh_odd[j][:], qhT[j][64:128, :])
                nc.sync.dma_start(kh_odd[j][:], khT[j][64:128, :])
            for hp in range(4):
                for hh in range(2):
                    h = 2 * hp + hh
                    if hh == 0:
                        qh = qhT[hp][0:64, :]
                        kh = khT[hp][0:64, :]
                    else:
                        qh = qh_odd[hp][:, :]
                        kh = kh_odd[hp][:, :]

                    # scores [k, q] -> exp -> eT tiles (for probs @ V)
                    eT = []
                    for kt in range(8):
                        psT = bigps.tile([128, S], F32, name="psT", tag="bigtile")
                        for qch in range(2):
                            nc.tensor.matmul(
                                psT[:, ts(qch, 512)],
                                (kh[:, ts(kt, 128)]),
                                (qh[:, ts(qch, 512)]),
                                start=True, stop=True,
                            )
                        et = peT.tile([128, S], F32R, name="eT", tag="eT")
                        nc.scalar.activation(et[:], psT[:], AF.Exp, scale=SCALE)
                        eT.append(et)

                    # scores [q, k] -> exp with row-sum accum -> e tiles
                    e_tiles = []
                    for qt in range(8):
                        psS = bigps.tile([128, S], F32, name="psS", tag="bigtile")
                        for kch in range(2):
                            nc.tensor.matmul(
                                psS[:, ts(kch, 512)],
                                (qh[:, ts(qt, 128)]),
                                (kh[:, ts(kch, 512)]),
                                start=True, stop=True,
                            )
                        et = pe_.tile([128, S], F32, name="e", tag="e")
                        idx = h * 8 + qt
                        nc.scalar.activation(
                            et[:], psS[:], AF.Exp, scale=SCALE,
                            accum_out=den_all[:, idx:idx + 1],
                        )
                        e_tiles.append(et)

                    # normalize p = e / den and store
                    nc.vector.reciprocal(
                        rec_all[:, h * 8:h * 8 + 8], den_all[:, h * 8:h * 8 + 8]
                    )
                    for qt in range(8):
                        pt = pp.tile([128, S], F32, name="p", tag="p")
                        nc.vector.tensor_scalar_mul(
                            pt[:], e_tiles[qt][:],
                            rec_all[:, h * 8 + qt:h * 8 + qt + 1],
                        )
                        nc.sync.dma_start(p_out[h, ts(qt, 128), :], pt[:])

                    # denominator as a [1, S] row via ones-matmul over eT
                    psden = bigps.tile([128, S], F32, name="psden", tag="bigtile")
                    for qch in range(2):
                        for kt in range(8):
                            nc.tensor.matmul(
                                psden[0:2, ts(qch, 512)],
                                (ones_col[:]),
                                (eT[kt][:, ts(qch, 512)]),
                                start=(kt == 0), stop=(kt == 7),
                            )
                    rec_row = pr.tile([1, S], F32, name="rec_row", tag="rec_row")
                    nc.vector.reciprocal(rec_row[0:1, :], psden[0:1, :])
                    recd = dscr.tile([1, S], F32, name="recd", tag="recd")
                    nc.sync.dma_start(recd[0:1, :], rec_row[0:1, :])
                    rbc = pr.tile([64, S], F32, name="rbc", tag="rbc")
                    nc.sync.dma_start(rbc[:, :], recd[0:1, :].to_broadcast((64, S)))

                    # ctx^T[d, q] = v_h^T @ eT (unnormalized), base-0 psum
                    psc = ctxps.tile([64, S], F32, name="psC", tag="psC")
                    for qch in range(2):
                        for kt in range(8):
                            nc.tensor.matmul(
                                psc[:, ts(qch, 512)],
                                (v_sb[kt][:, ts(h, 64)]),
                                (eT[kt][:, ts(qch, 512)]),
                                start=(kt == 0), stop=(kt == 7),
                            )
                    if hh == 0:
                        nc.vector.tensor_tensor(
                            ctxT[hp][0:64, :], psc[:, :], rbc[:], op=ALU.mult)
                    else:
                        csc = pcs.tile([64, S], F32R, name="csc", tag="csc")
                        nc.vector.tensor_tensor(
                            csc[:], psc[:, :], rbc[:], op=ALU.mult)
                        nc.sync.dma_start(ctxT[hp][64:128, :], csc[:])

        # ============ Phase E: output projection (partial) ============
        with tc.tile_pool(name="phE", bufs=1) as phe, \
                tc.tile_pool(name="phEo", bufs=2) as pheo:
            wot_sb = [phe.tile([128, E], F32R, name=f"wot{i}") for i in range(4)]
            for i in range(4):
                nc.sync.dma_start(wot_sb[i][:], wot[ts(i, 128), :])
            for st in range(8):
                pso = bigps.tile([128, E], F32, name="psO", tag="bigtile")
                for ech in range(2):
                    for kc in range(4):
                        nc.tensor.matmul(
                            pso[:, ts(ech, 512)],
                            (ctxT[kc][:, ts(st, 128)]),
                            (wot_sb[kc][:, ts(ech, 512)]),
                            start=(kc == 0), stop=(kc == 3),
                        )
                osb = pheo.tile([128, E], F32, name="osb", tag="osb")
                nc.vector.tensor_copy(osb[:], pso[:])
                nc.sync.dma_start(o_out[ts(st, 128), :], osb[:])

    nc.compile()
    return nc


_PROGRAM_CACHE = {}


def _get_program(cq, ck):
    key = (round(float(cq), 10), round(float(ck), 10), "f32r")
    if key not in _PROGRAM_CACHE:
        _PROGRAM_CACHE[key] = build_program(float(cq), float(ck))
    return _PROGRAM_CACHE[key]


def make_in_maps(inputs):
    """Host-side sharding: build the 8 per-core input dicts."""
    x = np.asarray(inputs["x"], np.float32)
    mask = np.asarray(inputs["mask"])
    in_maps = []
    for c in range(NCORES):
        b, g = divmod(c, 2)
        hs = slice(g * O, (g + 1) * O)
        maskf = mask[b].astype(np.float32)
        msum = max(float(maskf.sum()), 1e-9)
        bds = {nm: np.zeros((O, 97), np.float32)
               for nm in ("bdqA", "bdqB", "bdkA", "bdkB",
                          "bdq2A", "bdq2B", "bdk2A", "bdk2B")}
        for h in range(HPC):
            sfx, col = ("A", 32 * h) if h < 4 else ("B", 32 * (h - 4))
            rows = slice(h * D, (h + 1) * D)
            bds["bdq" + sfx][rows, col] = inputs["qg_wq"][0]
            bds["bdk" + sfx][rows, col] = inputs["kg_wq"][0]
            bds["bdq2" + sfx][rows, col] = inputs["qg_wk"][0]
            bds["bdk2" + sfx][rows, col] = inputs["kg_wk"][0]
        im = dict(
            xb=np.ascontiguousarray(x[:, b, :]),
            mfr=np.ascontiguousarray((maskf / msum)[None, :]),
            wqt=np.ascontiguousarray(inputs["Wq"][hs].T),
            wkt=np.ascontiguousarray(inputs["Wk"][hs].T),
            wvt=np.ascontiguousarray(inputs["Wv"][hs].T),
            wgqt=np.ascontiguousarray(inputs["Wgq"][hs].T),
            wgkt=np.ascontiguousarray(inputs["Wgk"][hs].T),
            bqc=np.ascontiguousarray(inputs["bq"][hs][:, None]),
            bkc=np.ascontiguousarray(inputs["bk"][hs][:, None]),
            bvr=np.ascontiguousarray(inputs["bv"][hs][None, :]),
            bgqc=np.ascontiguousarray(inputs["bgq"][hs][:, None]),
            bgkc=np.ascontiguousarray(inputs["bgk"][hs][:, None]),
            wot=np.ascontiguousarray(inputs["Wo"][:, hs].T),
            **bds,
        )
        im = {k: np.ascontiguousarray(v, np.float32) for k, v in im.items()}
        in_maps.append(im)
    return in_maps


def gather_outputs(inputs, results):
    out = np.zeros((S, B, E), np.float32)
    p_attn = np.empty((B, H, S, S), np.float32)
    bo = np.asarray(inputs["bo"], np.float32)
    for c in range(NCORES):
        b, g = divmod(c, 2)
        out[:, b, :] += results[c]["o_out"]
        p_attn[b, g * HPC:(g + 1) * HPC] = results[c]["p_out"]
    out += bo[None, None, :]
    return out, p_attn


def kernel(**inputs):
    cq = float(np.asarray(inputs["qg_bq"]).reshape(-1)[0]
               + np.asarray(inputs["qg_bk"]).reshape(-1)[0])
    ck = float(np.asarray(inputs["kg_bq"]).reshape(-1)[0]
               + np.asarray(inputs["kg_bk"]).reshape(-1)[0])
    nc = _get_program(cq, ck)
    in_maps = make_in_maps(inputs)
    res = run_bass_kernel_spmd(nc, in_maps, core_ids=list(range(NCORES)))
    return gather_outputs(inputs, res.results)


# revision 16
# speedup vs baseline: 1.1241x; 1.0313x over previous
"""Trainium2 Bass kernel for nn_ContextAttention (dense transformer block).

Sharding (8 NeuronCores): data-parallel over batch (B=4) x tensor-parallel
over heads (16 heads -> 2 groups of 8). Core c handles batch c//2, head
group c%2. Each core computes QKV projections for its 512 output dims,
the mean-pooled context projections, sequence gating, attention scores in
both orientations ([q,k] for the softmax/p_attn output and [k,q] for the
probs @ V contraction - avoids any on-chip transpose of the 1Mx8 prob
matrices), unnormalized softmax via ScalarE exp with fused row-sum
accumulation, and its slice of the output projection. The two cores of a
batch pair each emit a partial [S,E] output; the host sums the pair and
adds the output bias (the "all-reduce" of the output projection), and
concatenates the per-core [8,S,S] attention-prob slices.

Matmuls stream as float32r (fp32 data, fast PE mode); everything else fp32.
"""

import sys

for _p in ("/opt/trn_rl_repo", "/root/.axon_site/_ro/trn_rl_repo"):
    if _p not in sys.path:
        sys.path.append(_p)

from contextlib import ExitStack

import numpy as np

import concourse.bass as bass  # noqa: F401
import concourse.mybir as mybir
import concourse.tile as tile
from concourse import bacc
from concourse.bass import ts
from concourse.bass_utils import run_bass_kernel_spmd
from concourse.masks import make_identity

F32 = mybir.dt.float32
F32R = mybir.dt.float32r
BF = mybir.dt.bfloat16
AF = mybir.ActivationFunctionType
ALU = mybir.AluOpType

S = 1024   # sequence length
B = 4      # batch
E = 1024   # embed dim
H = 16     # total heads
D = 64     # head dim
HPC = 8    # heads per core
O = HPC * D  # 512 local projection dims per core
NCORES = 8
SCALE = 0.125  # 1/sqrt(D)



def build_program(cq: float, ck: float):
    """Build the single SPMD Bass program (same program, per-core data)."""
    nc = bacc.Bacc(
        "TRN2",
        target_bir_lowering=False,
        debug=False,
        num_devices=NCORES,
    )

    # ---- DRAM I/O (per-core data) ----
    xb = nc.dram_tensor("xb", [S, E], F32, kind="ExternalInput").ap()
    mfr = nc.dram_tensor("mfr", [1, S], F32, kind="ExternalInput").ap()
    wqt = nc.dram_tensor("wqt", [E, O], F32R, kind="ExternalInput").ap()
    wkt = nc.dram_tensor("wkt", [E, O], F32R, kind="ExternalInput").ap()
    wvt = nc.dram_tensor("wvt", [E, O], F32R, kind="ExternalInput").ap()
    wgqt = nc.dram_tensor("wgqt", [E, O], F32R, kind="ExternalInput").ap()
    wgkt = nc.dram_tensor("wgkt", [E, O], F32R, kind="ExternalInput").ap()
    bqc = nc.dram_tensor("bqc", [O, 1], F32, kind="ExternalInput").ap()
    bkc = nc.dram_tensor("bkc", [O, 1], F32, kind="ExternalInput").ap()
    bvr = nc.dram_tensor("bvr", [1, O], F32, kind="ExternalInput").ap()
    bgqc = nc.dram_tensor("bgqc", [O, 1], F32, kind="ExternalInput").ap()
    bgkc = nc.dram_tensor("bgkc", [O, 1], F32, kind="ExternalInput").ap()
    wot = nc.dram_tensor("wot", [O, E], F32R, kind="ExternalInput").ap()
    bd_in = {nm: nc.dram_tensor(nm, [O, 97], F32R, kind="ExternalInput").ap()
             for nm in ("bdqA", "bdqB", "bdkA", "bdkB",
                        "bdq2A", "bdq2B", "bdk2A", "bdk2B")}

    p_out = nc.dram_tensor("p_out", [HPC, S, S], F32, kind="ExternalOutput").ap()
    o_out = nc.dram_tensor("o_out", [S, E], F32, kind="ExternalOutput").ap()

    with tile.TileContext(nc) as tc, ExitStack() as stk:
        persist = stk.enter_context(tc.tile_pool(name="persist", bufs=1))
        bigps = stk.enter_context(tc.tile_pool(name="bigps", bufs=2, space="PSUM"))
        dscr = stk.enter_context(tc.tile_pool(name="dscr", bufs=2, space="DRAM"))

        ident = persist.tile([128, 128], F32, name="ident")
        make_identity(nc, ident[:])

        # ============ Phase A: load x, transpose, mean-pool g ============
        g_sb = persist.tile([128, 8], F32R, name="g_sb")
        g_f32 = persist.tile([128, 8], F32, name="g_f32")
        v_sb = [persist.tile([128, O], F32R, name=f"v{i}") for i in range(8)]
        gq_sb = persist.tile([128, 4], F32R, name="gq_sb")
        gk_sb = persist.tile([128, 4], F32R, name="gk_sb")
        qd_sb = [persist.tile([128, 1], F32, name=f"qd_sb{i}") for i in range(2)]
        kd_sb = [persist.tile([128, 1], F32, name=f"kd_sb{i}") for i in range(2)]
        bd_sb = {nm: [persist.tile([128, 97], F32R, name=f"{nm}_{i}")
                      for i in range(4)]
                 for nm in ("bdqA", "bdqB", "bdkA", "bdkB")}

        qhT = [persist.tile([128, S], F32R, name=f"qhT{j}") for j in range(4)]
        khT = [persist.tile([128, S], F32R, name=f"khT{j}") for j in range(4)]

        with tc.tile_pool(name="qkvp", bufs=1) as qkvp:
            qT = [qkvp.tile([128, S], F32R, name=f"qT{j}") for j in range(4)]
            kT = [qkvp.tile([128, S], F32R, name=f"kT{j}") for j in range(4)]

            with tc.tile_pool(name="xtp", bufs=1) as xtp:
                xT = [xtp.tile([128, S], F32R, name=f"xT{j}") for j in range(8)]

                with tc.tile_pool(name="phA", bufs=8) as pha, \
                        tc.tile_pool(name="phA1", bufs=1) as pha1:
                    x_nat = []
                    for i in range(8):
                        t = pha.tile([128, E], F32, name="x_nat", tag="x_nat")
                        nc.scalar.dma_start(t[:], xb[ts(i, 128), :])
                        x_nat.append(t)
                    mf_bc = pha1.tile([128, S], F32, name="mf_bc")
                    nc.sync.dma_start(mf_bc[:], mfr[0:1, :].to_broadcast((128, S)))

                    for j in range(8):
                        psx = bigps.tile([128, S], F32, name="psx", tag="bigtile")
                        for i in range(8):
                            nc.tensor.transpose(
                                psx[:, ts(i, 128)], x_nat[i][:, ts(j, 128)], ident[:]
                            )
                        nc.vector.tensor_copy(xT[j][:], psx[:])
                    gscr = pha1.tile([128, S], F32, name="gscr")
                    for j in range(8):
                        nc.vector.scalar_tensor_tensor(
                            gscr[:], xT[j][:].bitcast(F32), 1.0, mf_bc[:],
                            op0=ALU.mult, op1=ALU.mult,
                            accum_out=g_f32[:, j:j + 1],
                        )
                    nc.vector.tensor_copy(g_sb[:], g_f32[:])

                # ======== Phase B: projections ========
                with tc.tile_pool(name="phB", bufs=1) as phb, \
                        tc.tile_pool(name="smallps", bufs=2,
                                     space="PSUM") as smallps:
                    bq_sb = phb.tile([128, 4], F32, name="bq_sb")
                    bk_sb = phb.tile([128, 4], F32, name="bk_sb")
                    bgq_sb = phb.tile([128, 4], F32, name="bgq_sb")
                    bgk_sb = phb.tile([128, 4], F32, name="bgk_sb")
                    for oc in range(4):
                        nc.scalar.dma_start(bq_sb[:, oc:oc + 1], bqc[ts(oc, 128), :])
                        nc.scalar.dma_start(bk_sb[:, oc:oc + 1], bkc[ts(oc, 128), :])
                        nc.scalar.dma_start(bgq_sb[:, oc:oc + 1], bgqc[ts(oc, 128), :])
                        nc.scalar.dma_start(bgk_sb[:, oc:oc + 1], bgkc[ts(oc, 128), :])
                    bv_sb = phb.tile([1, O], F32, name="bv_sb")
                    nc.scalar.dma_start(bv_sb[:], bvr[:, :])
                    ones1 = phb.tile([1, 128], F32, name="ones1")
                    nc.vector.memset(ones1[:], 1.0)
                    for nm in ("bdqA", "bdqB", "bdkA", "bdkB"):
                        for i in range(4):
                            nc.scalar.dma_start(bd_sb[nm][i][:], bd_in[nm][ts(i, 128), :])
                    bd2_sb = {nm: [phb.tile([128, 97], F32R, name=f"{nm}_{i}")
                                   for i in range(4)]
                              for nm in ("bdq2A", "bdq2B", "bdk2A", "bdk2B")}
                    for nm in ("bdq2A", "bdq2B", "bdk2A", "bdk2B"):
                        for i in range(4):
                            nc.scalar.dma_start(bd2_sb[nm][i][:], bd_in[nm][ts(i, 128), :])

                    def load_wset(src_ap):
                        tiles = []
                        for ke in range(8):
                            t = phb.tile([128, O], F32R, name="wmat", tag="wmat",
                                         bufs=16)
                            nc.scalar.dma_start(t[:], src_ap[ts(ke, 128), :])
                            tiles.append(t)
                        return tiles

                    # q^T, k^T projections: [O, S] as 4 tiles [128, 1024]
                    for dst, wsrc, bias in ((qT, wqt, bq_sb), (kT, wkt, bk_sb)):
                        wsb = load_wset(wsrc)
                        for oc in range(4):
                            ps = bigps.tile([128, S], F32, name="psB", tag="bigtile")
                            for tch in range(2):
                                for ke in range(8):
                                    nc.tensor.matmul(
                                        ps[:, ts(tch, 512)],
                                        (wsb[ke][:, ts(oc, 128)]),
                                        (xT[ke][:, ts(tch, 512)]),
                                        start=(ke == 0), stop=(ke == 7),
                                    )
                            nc.vector.tensor_scalar_add(dst[oc][:], ps[:], bias[:, oc:oc + 1])

                    # v: [S, O] natural layout, bias folded via rank-1 matmul
                    wsb = load_wset(wvt)
                    for tt in range(8):
                        psv = bigps.tile([128, O], F32, name="psv", tag="bigtile")
                        for ke in range(8):
                            nc.tensor.matmul(
                                psv[:, :],
                                (xT[ke][:, ts(tt, 128)]),
                                (wsb[ke][:]),
                                start=(ke == 0), stop=False,
                            )
                        nc.tensor.matmul(
                            psv[:, :], ones1[:], bv_sb[:],
                            start=False, stop=True,
                        )
                        nc.vector.tensor_copy(v_sb[tt][:], psv[:, :])

                    # context projections gq, gk: [O, 1]
                    for dst, wsrc, bias in ((gq_sb, wgqt, bgq_sb), (gk_sb, wgkt, bgk_sb)):
                        wsb = load_wset(wsrc)
                        for oc in range(4):
                            psg = smallps.tile([128, 1], F32, name="psg", tag="smalltile")
                            for ke in range(8):
                                nc.tensor.matmul(
                                    psg[:, :],
                                    wsb[ke][:, ts(oc, 128)].bitcast(F32),
                                    g_sb[:, ke:ke + 1].bitcast(F32),
                                    start=(ke == 0), stop=(ke == 7),
                                )
                            nc.vector.tensor_scalar_add(
                                dst[:, oc:oc + 1], psg[:, :], bias[:, oc:oc + 1]
                            )

                    # per-head gate constants (heads spread to partitions 0/32/64/96)
                    for dsts, nms, gcol, const in (
                        (qd_sb, ("bdq2A", "bdq2B"), gq_sb, cq),
                        (kd_sb, ("bdk2A", "bdk2B"), gk_sb, ck),
                    ):
                        for grp in range(2):
                            psd = smallps.tile([128, 1], F32, name="psd", tag="smalltile")
                            for kc in range(4):
                                nc.tensor.matmul(
                                    psd[0:97, :],
                                    bd2_sb[nms[grp]][kc][:].bitcast(F32),
                                    gcol[:, kc:kc + 1].bitcast(F32),
                                    start=(kc == 0), stop=(kc == 3),
                                )
                            nc.vector.tensor_scalar_add(
                                dsts[grp][0:97, :], psd[0:97, :], float(const))

            # ======== Phase C: sequence gating (xT freed) ========
            with tc.tile_pool(name="phC", bufs=2) as phc, \
                    tc.tile_pool(name="phCa", bufs=2) as phca:
                for src, dst, nms, gcol, dcols in (
                    (qT, qhT, ("bdqA", "bdqB"), gq_sb, qd_sb),
                    (kT, khT, ("bdkA", "bdkB"), gk_sb, kd_sb),
                ):
                    alphas = []
                    for grp in range(2):
                        psa = bigps.tile([128, S], F32, name="psa", tag="bigtile")
                        for tch in range(2):
                            for kc in range(4):
                                nc.tensor.matmul(
                                    psa[0:97, ts(tch, 512)],
                                    (bd_sb[nms[grp]][kc][:]),
                                    (src[kc][:, ts(tch, 512)]),
                                    start=(kc == 0), stop=(kc == 3),
                                )
                        alpha = phca.tile([128, S], F32, name="alpha", tag="alpha")
                        nc.scalar.activation(alpha[0:97, :], psa[0:97, :], AF.Sigmoid,
                                             bias=dcols[grp][0:97, :])
                        alphas.append(alpha)
                    alpha_d = dscr.tile([8, S], F32, name="alpha_d", tag="alpha_d")
                    for grp in range(2):
                        nc.sync.dma_start(alpha_d[4 * grp:4 * grp + 4, :],
                                          alphas[grp][0:128:32, :])
                    for j in range(4):
                        abc = phc.tile([128, S], F32, name="abc", tag="abc")
                        for hh in range(2):
                            h = 2 * j + hh
                            nc.sync.dma_start(
                                abc[64 * hh:64 * hh + 64, :],
                                alpha_d[h:h + 1, :].to_broadcast((64, S)))
                        tmpg = phc.tile([128, S], F32, name="tmpg", tag="tmpg")
                        nc.vector.scalar_tensor_tensor(
                            tmpg[:], src[j][:].bitcast(F32), gcol[:, j:j + 1].bitcast(F32), abc[:],
                            op0=ALU.subtract, op1=ALU.mult,
                        )
                        nc.vector.tensor_sub(dst[j][:], src[j][:].bitcast(F32), tmpg[:])

        # ============ Phase D: attention per head pair ============
        den_all = persist.tile([128, 8 * HPC], F32, name="den_all")
        rec_all = persist.tile([128, 8 * HPC], F32, name="rec_all")
        ctxT = [persist.tile([128, S], F32R, name=f"ctxT{j}") for j in range(4)]
        # fp32r matmuls require partition-base-0 operands: stage odd heads
        # (partitions 64..127 of the packed q/k tiles) into base-0 tiles.
        with tc.tile_pool(name="phD_qk", bufs=1) as pqk, \
                tc.tile_pool(name="ctxps", bufs=1, space="PSUM") as ctxps, \
                tc.tile_pool(name="dps", bufs=2, space="PSUM") as dps, \
                tc.tile_pool(name="phD_eT", bufs=9) as peT, \
                tc.tile_pool(name="phD_e", bufs=8) as pe_, \
                tc.tile_pool(name="phD_p", bufs=3) as pp, \
                tc.tile_pool(name="phD_r", bufs=2) as pr, \
                tc.tile_pool(name="phD_cs", bufs=1) as pcs:
            qh_odd = [pqk.tile([64, S], F32R, name=f"qho{j}") for j in range(4)]
            kh_odd = [pqk.tile([64, S], F32R, name=f"kho{j}") for j in range(4)]
            for j in range(4):
                nc.sync.dma_start(qh_odd[j][:], qhT[j][64:128, :])
                nc.sync.dma_start(kh_odd[j][:], khT[j][64:128, :])
            # bf16 copies for the probs@V side (scores for p_attn stay fp32r)
            qhB = [pqk.tile([128, S], BF, name=f"qhB{j}") for j in range(4)]
            khB = [pqk.tile([128, S], BF, name=f"khB{j}") for j in range(4)]
            vB = [pqk.tile([128, O], BF, name=f"vB{i}") for i in range(8)]
            for j in range(4):
                nc.vector.tensor_copy(qhB[j][:], qhT[j][:].bitcast(F32))
                nc.vector.tensor_copy(khB[j][:], khT[j][:].bitcast(F32))
            for i in range(8):
                nc.vector.tensor_copy(vB[i][:], v_sb[i][:].bitcast(F32))
            for hp in range(4):
                for hh in range(2):
                    h = 2 * hp + hh
                    if hh == 0:
                        qh = qhT[hp][0:64, :]
                        kh = khT[hp][0:64, :]
                    else:
                        qh = qh_odd[hp][:, :]
                        kh = kh_odd[hp][:, :]
                    qhb = qhB[hp][64 * hh:64 * hh + 64, :]
                    khb = khB[hp][64 * hh:64 * hh + 64, :]

                    # scores [k, q] -> exp -> eT tiles (for probs @ V)
                    eT = []
                    for kt in range(8):
                        psT = bigps.tile([128, S], F32, name="psT", tag="bigtile")
                        for qch in range(2):
                            nc.tensor.matmul(
                                psT[:, ts(qch, 512)],
                                (khb[:, ts(kt, 128)]),
                                (qhb[:, ts(qch, 512)]),
                                start=True, stop=True,
                            )
                        et = peT.tile([128, S], BF, name="eT", tag="eT")
                        nc.scalar.activation(et[:], psT[:], AF.Exp, scale=SCALE)
                        eT.append(et)

                    # scores [q, k] -> exp with row-sum accum -> e tiles
                    e_tiles = []
                    for qt in range(8):
                        psS = bigps.tile([128, S], F32, name="psS", tag="bigtile")
                        for kch in range(2):
                            nc.tensor.matmul(
                                psS[:, ts(kch, 512)],
                                (qh[:, ts(qt, 128)]),
                                (kh[:, ts(kch, 512)]),
                                start=True, stop=True,
                            )
                        et = pe_.tile([128, S], F32, name="e", tag="e")
                        idx = h * 8 + qt
                        nc.scalar.activation(
                            et[:], psS[:], AF.Exp, scale=SCALE,
                            accum_out=den_all[:, idx:idx + 1],
                        )
                        e_tiles.append(et)

                    # normalize p = e / den and store
                    nc.vector.reciprocal(
                        rec_all[:, h * 8:h * 8 + 8], den_all[:, h * 8:h * 8 + 8]
                    )
                    for qt in range(8):
                        pt = pp.tile([128, S], F32, name="p", tag="p")
                        nc.vector.tensor_scalar_mul(
                            pt[:], e_tiles[qt][:],
                            rec_all[:, h * 8 + qt:h * 8 + qt + 1],
                        )
                        nc.sync.dma_start(p_out[h, ts(qt, 128), :], pt[:])

                    # reciprocal row: transpose rec_all[:, h-block] -> [8,128]
                    psrt = dps.tile([8, 128], F32, name="psrt", tag="psrt")
                    nc.tensor.transpose(
                        psrt[:], rec_all[:, h * 8:h * 8 + 8], ident[:])
                    rec8 = pr.tile([8, 128], F32, name="rec8", tag="rec8")
                    nc.vector.tensor_copy(rec8[:], psrt[:])
                    recd = dscr.tile([1, S], F32, name="recd", tag="recd")
                    recd8 = recd[0:1, :].rearrange("a (c d) -> (a c) d", c=8)
                    nc.sync.dma_start(recd8, rec8[:])
                    rbc = pr.tile([64, S], F32, name="rbc", tag="rbc")
                    nc.sync.dma_start(rbc[:, :], recd[0:1, :].to_broadcast((64, S)))

                    # ctx^T[d, q] = v_h^T @ eT (unnormalized), base-0 psum
                    psc = ctxps.tile([64, S], F32, name="psC", tag="psC")
                    for qch in range(2):
                        for kt in range(8):
                            nc.tensor.matmul(
                                psc[:, ts(qch, 512)],
                                (vB[kt][:, ts(h, 64)]),
                                (eT[kt][:, ts(qch, 512)]),
                                start=(kt == 0), stop=(kt == 7),
                            )
                    if hh == 0:
                        nc.vector.tensor_tensor(
                            ctxT[hp][0:64, :], psc[:, :], rbc[:], op=ALU.mult)
                    else:
                        csc = pcs.tile([64, S], F32R, name="csc", tag="csc")
                        nc.vector.tensor_tensor(
                            csc[:], psc[:, :], rbc[:], op=ALU.mult)
                        nc.sync.dma_start(ctxT[hp][64:128, :], csc[:])

        # ============ Phase E: output projection (partial) ============
        with tc.tile_pool(name="phE", bufs=1) as phe, \
                tc.tile_pool(name="phEo", bufs=2) as pheo:
            wot_sb = [phe.tile([128, E], F32R, name=f"wot{i}") for i in range(4)]
            for i in range(4):
                nc.scalar.dma_start(wot_sb[i][:], wot[ts(i, 128), :])
            for st in range(8):
                pso = bigps.tile([128, E], F32, name="psO", tag="bigtile")
                for ech in range(2):
                    for kc in range(4):
                        nc.tensor.matmul(
                            pso[:, ts(ech, 512)],
                            (ctxT[kc][:, ts(st, 128)]),
                            (wot_sb[kc][:, ts(ech, 512)]),
                            start=(kc == 0), stop=(kc == 3),
                        )
                osb = pheo.tile([128, E], F32, name="osb", tag="osb")
                nc.vector.tensor_copy(osb[:], pso[:])
                nc.sync.dma_start(o_out[ts(st, 128), :], osb[:])

    nc.compile()
    return nc


_PROGRAM_CACHE = {}


def _get_program(cq, ck):
    key = (round(float(cq), 10), round(float(ck), 10), "f32r")
    if key not in _PROGRAM_CACHE:
        _PROGRAM_CACHE[key] = build_program(float(cq), float(ck))
    return _PROGRAM_CACHE[key]


def make_in_maps(inputs):
    """Host-side sharding: build the 8 per-core input dicts."""
    x = np.asarray(inputs["x"], np.float32)
    mask = np.asarray(inputs["mask"])
    in_maps = []
    for c in range(NCORES):
        b, g = divmod(c, 2)
        hs = slice(g * O, (g + 1) * O)
        maskf = mask[b].astype(np.float32)
        msum = max(float(maskf.sum()), 1e-9)
        bds = {nm: np.zeros((O, 97), np.float32)
               for nm in ("bdqA", "bdqB", "bdkA", "bdkB",
                          "bdq2A", "bdq2B", "bdk2A", "bdk2B")}
        for h in range(HPC):
            sfx, col = ("A", 32 * h) if h < 4 else ("B", 32 * (h - 4))
            rows = slice(h * D, (h + 1) * D)
            bds["bdq" + sfx][rows, col] = inputs["qg_wq"][0]
            bds["bdk" + sfx][rows, col] = inputs["kg_wq"][0]
            bds["bdq2" + sfx][rows, col] = inputs["qg_wk"][0]
            bds["bdk2" + sfx][rows, col] = inputs["kg_wk"][0]
        im = dict(
            xb=np.ascontiguousarray(x[:, b, :]),
            mfr=np.ascontiguousarray((maskf / msum)[None, :]),
            wqt=np.ascontiguousarray(inputs["Wq"][hs].T),
            wkt=np.ascontiguousarray(inputs["Wk"][hs].T),
            wvt=np.ascontiguousarray(inputs["Wv"][hs].T),
            wgqt=np.ascontiguousarray(inputs["Wgq"][hs].T),
            wgkt=np.ascontiguousarray(inputs["Wgk"][hs].T),
            bqc=np.ascontiguousarray(inputs["bq"][hs][:, None]),
            bkc=np.ascontiguousarray(inputs["bk"][hs][:, None]),
            bvr=np.ascontiguousarray(inputs["bv"][hs][None, :]),
            bgqc=np.ascontiguousarray(inputs["bgq"][hs][:, None]),
            bgkc=np.ascontiguousarray(inputs["bgk"][hs][:, None]),
            wot=np.ascontiguousarray(inputs["Wo"][:, hs].T),
            **bds,
        )
        im = {k: np.ascontiguousarray(v, np.float32) for k, v in im.items()}
        in_maps.append(im)
    return in_maps


def gather_outputs(inputs, results):
    out = np.zeros((S, B, E), np.float32)
    p_attn = np.empty((B, H, S, S), np.float32)
    bo = np.asarray(inputs["bo"], np.float32)
    for c in range(NCORES):
        b, g = divmod(c, 2)
        out[:, b, :] += results[c]["o_out"]
        p_attn[b, g * HPC:(g + 1) * HPC] = results[c]["p_out"]
    out += bo[None, None, :]
    return out, p_attn


def kernel(**inputs):
    cq = float(np.asarray(inputs["qg_bq"]).reshape(-1)[0]
               + np.asarray(inputs["qg_bk"]).reshape(-1)[0])
    ck = float(np.asarray(inputs["kg_bq"]).reshape(-1)[0]
               + np.asarray(inputs["kg_bk"]).reshape(-1)[0])
    nc = _get_program(cq, ck)
    in_maps = make_in_maps(inputs)
    res = run_bass_kernel_spmd(nc, in_maps, core_ids=list(range(NCORES)))
    return gather_outputs(inputs, res.results)
